# revision 1
# baseline (speedup 1.0000x reference)
"""BiDiTreeLSTM Trainium2 kernel.

Full-input contract: kernel(**inputs) takes the unsharded numpy inputs of
reference.setup_inputs() and returns the full [64, 512] output.

Strategy: data-parallel over trees (8 trees per NeuronCore, 8 cores).
Per-core layout is feature-major: every node-state tensor lives in SBUF as
[128 partitions, 2 feature-chunk column halves] ("g-major"), where within a
half the columns are level-major blocks, tree-major within a level.  With
that ordering the two children of parent column c in level l are columns 2c
and 2c+1 of level l+1, so child gather/scatter is pure stride-2 APs.

All matmuls contract the feature dimension (SBUF partition axis) using
float32r (full-rate fp32 mode for moving dim >= 256; ~tf32 rounding).
Matmul-feeding tiles are float32r-typed so producers emit rounded values
(BIR verifier rule).

Exploited zero-fills from the problem spec (verified against the reference
in test.py): h0 == 0, c0 == 0, and all four bias vectors == 0.  The
recurrence starts from zero, and gate pre-activations skip the bias add.

Perf structure (from NTFF profiles): the PE/LDWEIGHTS queue is the
bottleneck.  The W-parts of the small levels (l<6, <=256 cols) are
precomputed in two batched matmuls over the contiguous first SM columns;
gates use wide [128, 2T] PSUM tiles evacuated immediately by the
activations (fast bank recycling); each tile's accumulation groups open
with the input-independent W-matmuls and close with the U-matmuls so PE
always has work while the hsum/f chains run; hsum is hoisted per level
(small levels on the otherwise-idle GpSimd).
"""

import numpy as np

B, NN, XS, H = 64, 1023, 256, 256
NCORES = 8
DEPTH = 9  # levels 0..9, level l has 2^l nodes per tree
TMAX = 512
SM_LEV = 6  # levels 0..SM_LEV-1 get batched W-projections

_CACHE = {}

LAST_EXEC_NS = None


def _levels(bl):
    levw = [bl * (1 << l) for l in range(DEPTH + 1)]
    levo = [bl * ((1 << l) - 1) for l in range(DEPTH + 1)]
    tot = bl * NN
    return levw, levo, tot


def _build_nc(bl):
    from concourse import bacc
    import concourse.mybir as mybir
    import concourse.tile as tile

    f32 = mybir.dt.float32
    f32r = mybir.dt.float32r
    Sig = mybir.ActivationFunctionType.Sigmoid
    Tanh = mybir.ActivationFunctionType.Tanh

    LEVW, LEVO, TOT = _levels(bl)
    SM = LEVO[SM_LEV]  # cols of levels 0..SM_LEV-1 (contiguous, level-major)

    nc = bacc.Bacc("TRN2", target_bir_lowering=False)

    xT_d = nc.declare_dram_parameter("xT", [XS, TOT], f32, isOutput=False)
    w_iou_bu_d = nc.declare_dram_parameter("w_iou_bu_T", [XS, 3 * H], f32, isOutput=False)
    u_iou_bu_d = nc.declare_dram_parameter("u_iou_bu_T", [H, 3 * H], f32, isOutput=False)
    u_f_bu_d = nc.declare_dram_parameter("u_f_bu_T", [H, H], f32, isOutput=False)
    wx_td_d = nc.declare_dram_parameter("wx_iou_td_T", [XS, 3 * H], f32, isOutput=False)
    wh_td_d = nc.declare_dram_parameter("wh_iou_td_T", [H, 3 * H], f32, isOutput=False)
    u_iou_td_d = nc.declare_dram_parameter("u_iou_td_T", [H, 3 * H], f32, isOutput=False)
    u_f_td_d = nc.declare_dram_parameter("u_f_td_T", [H, H], f32, isOutput=False)
    out_d = nc.declare_dram_parameter("out", [512, bl], f32, isOutput=True)

    with tile.TileContext(nc) as tc:
        with (
            tc.tile_pool(name="const", bufs=1) as const,
            tc.tile_pool(name="hbu_pool", bufs=1) as hbu_pool,
            tc.tile_pool(name="work", bufs=2) as work,
            tc.tile_pool(name="xtp", bufs=2) as xtp,
            tc.tile_pool(name="psg", bufs=3, space="PSUM") as psg,
            tc.tile_pool(name="psf", bufs=1, space="PSUM") as psf,
        ):
            # ---- weights (lhsT layout [in_feat, out_feat]); td tiles rotate
            # into the bu slots after the bu pass releases them ----
            def load_w(dram, cols, nm):
                ts = []
                for k in (0, 1):
                    tag, nb = ("w768", 6) if cols == 768 else ("uf", 2)
                    t = const.tile([128, cols], f32r, name=f"{nm}{k}", tag=tag, bufs=nb)
                    # weight loads go on the Scalar HWDGE queue so they don't
                    # serialize behind the xt streaming loads on Sync
                    nc.scalar.dma_start(
                        out=t, in_=dram[k * 128:(k + 1) * 128, :].bitcast(f32r)
                    )
                    ts.append(t)
                return ts

            w_bu = load_w(w_iou_bu_d, 3 * H, "wbu")
            u_bu = uf_bu = None  # loaded lazily once the leaf level is emitted

            hbu = hbu_pool.tile([128, 2 * TOT], f32r, name="hbu", tag="hbu")
            mean = const.tile([128, 2, bl], f32, name="mean", tag="mean")

            # X^T for the small levels, kept for both precompute passes
            # (DMA'd lazily right before the first small level needs it)
            xsm = const.tile([128, 2 * SM], f32r, name="xsm", tag="xsm")

            def load_xsm():
                for k in (0, 1):
                    nc.scalar.dma_start(
                        out=xsm[:, k * SM:(k + 1) * SM],
                        in_=xT_d[k * 128:(k + 1) * 128, 0:SM].bitcast(f32r),
                    )

            def load_x(off, o0, T):
                xt = xtp.tile([128, 2 * T], f32r, name="xt", tag="xt", bufs=3)
                for k in (0, 1):
                    nc.sync.dma_start(
                        out=xt[:, k * T:(k + 1) * T],
                        in_=xT_d[
                            k * 128:(k + 1) * 128, off + o0:off + o0 + T
                        ].bitcast(f32r),
                    )
                return xt

            def g2(ap, width):
                return ap.rearrange("p (g c) -> p g c", g=2)

            def iou_mms(T, phase1, phase2=None):
                """Allocate the 3 gate psum tiles and emit phase1 matmuls.
                When T == 512 the two g-halves sit in separate PSUM banks
                (zero regions), so phase2 can be deferred with the
                accumulation groups left open -- PE then has independent W
                work while the f/hsum chains run; close() emits phase2.
                For T < 512 both halves share a zero region, so everything
                is emitted merged up front and close() is a no-op."""
                merged = phase2 is not None and T < 512
                p1 = phase1 + phase2 if merged else phase1
                pending = phase2 is not None and not merged
                pg = {}
                for gi, gate in enumerate(("i", "o", "u")):
                    p = psg.tile([128, 2 * T], f32, name=f"pg{gate}", tag="pg")
                    pg[gate] = p
                    for g in (0, 1):
                        ms = slice((2 * gi + g) * 128, (2 * gi + g + 1) * 128)
                        mms = [
                            (pair[k][:, ms], rhs(k))
                            for pair, rhs in p1
                            for k in (0, 1)
                        ]
                        for i, (lhs, rhs) in enumerate(mms):
                            nc.tensor.matmul(
                                p[:, g * T:(g + 1) * T],
                                lhs,
                                rhs,
                                start=(i == 0),
                                stop=(not pending and i == len(mms) - 1),
                            )

                def close():
                    if not pending:
                        return
                    for gi2 in range(3):
                        p = pg[("i", "o", "u")[gi2]]
                        for g in (0, 1):
                            ms = slice((2 * gi2 + g) * 128, (2 * gi2 + g + 1) * 128)
                            mms = [
                                (pair[k][:, ms], rhs(k))
                                for pair, rhs in phase2
                                for k in (0, 1)
                            ]
                            for i, (lhs, rhs) in enumerate(mms):
                                nc.tensor.matmul(
                                    p[:, g * T:(g + 1) * T],
                                    lhs,
                                    rhs,
                                    start=False,
                                    stop=(i == len(mms) - 1),
                                )

                return pg, close

            def precompute(rhs_for):
                """pre[:, m, :] = sum over (lhsT_pair, rhs_slicer): lhsT.T @ rhs
                over the SM small-level columns."""
                pre = const.tile([128, 6, SM], f32, name="pre", tag="pre")
                for m in range(6):
                    p = psg.tile([128, SM], f32, name="pp", tag="pg")
                    mms = []
                    for pair, rhs in rhs_for:
                        for k in (0, 1):
                            mms.append((pair[k][:, m * 128:(m + 1) * 128], rhs(k)))
                    for i, (lhs, rhs) in enumerate(mms):
                        nc.tensor.matmul(
                            p, lhs, rhs, start=(i == 0), stop=(i == len(mms) - 1)
                        )
                    nc.vector.tensor_copy(pre[:, m, :], p)
                return pre

            def gates(pg, T, c_red, c_out, h_out, leaf_sink=None, pre=None, lev=0):
                """pg: dict gate->psum tile [128, 2T] (g-major halves).
                c_red: None | ("full", ap[128,2,T]) | ("parent", ap[128,2,pT])
                c_out/h_out: [128, 2, T] views; leaf_sink(ht) for td leaves."""
                pgi, pgo, pgu = pg["i"], pg["o"], pg["u"]
                if pre is not None:
                    # add the precomputed W-part into the U-part psum
                    for gi, p in enumerate((pgi, pgo, pgu)):
                        pv = g2(p, T)
                        prv = pre[:, 2 * gi:2 * gi + 2, LEVO[lev]:LEVO[lev] + T]
                        nc.vector.tensor_add(pv, pv, prv)
                # Evacuate each gate from PSUM at the activation step so the
                # banks recycle fast (PE stalls on bank WAR otherwise), and
                # gate elementwise ops run SBUF-only (DVE 2x mode).
                # all three PSUM-reading activations first: the gate banks
                # recycle before the serial c/h chain runs
                si = work.tile([128, 2 * T], f32, name="si", tag="ga")
                nc.scalar.activation(si, pgi, Sig)
                so = work.tile([128, 2 * T], f32, name="so", tag="gb", bufs=3)
                nc.scalar.activation(so, pgo, Sig)
                tu = work.tile([128, 2 * T], f32, name="tu", tag="gb", bufs=3)
                nc.scalar.activation(tu, pgu, Tanh)
                if c_red is None:
                    nc.vector.tensor_mul(c_out, g2(si, T), g2(tu, T))
                else:
                    nc.vector.tensor_mul(si, si, tu)  # situ, in place
                    kind, cr = c_red
                    if kind == "full":
                        nc.vector.tensor_add(c_out, g2(si, T), cr)
                    else:  # parent-granularity c_red, broadcast to child pairs
                        pT = T // 2
                        si4 = si.rearrange("p (g n two) -> p g n two", g=2, two=2)
                        co4 = c_out.rearrange("p g (n two) -> p g n two", two=2)
                        crb = cr.to_broadcast([128, 2, pT, 2])
                        nc.vector.tensor_add(co4, si4, crb)
                tct = work.tile([128, 2 * T], f32, name="tct", tag="ga")
                nc.scalar.activation(g2(tct, T), c_out, Tanh)
                if h_out is not None:
                    nc.vector.tensor_mul(h_out, g2(so, T), g2(tct, T))
                else:
                    ht = work.tile([128, 2 * T], f32, name="ht", tag="hsum", bufs=3)
                    nc.vector.tensor_mul(g2(ht, T), g2(so, T), g2(tct, T))
                    leaf_sink(ht)

            # ================= bottom-up =================
            pre_bu = None
            with tc.tile_pool(name="bu_state", bufs=1) as bu_state:
                c_next = None
                C_next = 0
                for l in range(DEPTH, -1, -1):
                    if l == SM_LEV - 1 and pre_bu is None:
                        load_xsm()
                        pre_bu = precompute(
                            [(w_bu, lambda k: xsm[:, k * SM:(k + 1) * SM])]
                        )
                    if l == DEPTH - 1 and u_bu is None:
                        u_bu = load_w(u_iou_bu_d, 3 * H, "ubu")
                        uf_bu = load_w(u_f_bu_d, H, "ufbu")
                    C, off = LEVW[l], LEVO[l]
                    T = min(TMAX, C)
                    leaf = l == DEPTH
                    small = l < SM_LEV
                    par = "A" if l % 2 else "Bp"
                    c_cur = bu_state.tile(
                        [128, 2 * C], f32, name=f"c{l}", tag=f"c{par}"
                    )
                    choff = LEVO[l + 1] if not leaf else 0
                    ntile = C // T
                    # hsum for the whole level up front: it only needs the
                    # previous level's h, and putting it first in the DVE
                    # queue keeps the iou U-matmuls from waiting behind the
                    # previous tile's situ/c/h chain
                    hsums = []
                    if not leaf:
                        for j in range(ntile):
                            o0 = j * T
                            ncj = 2 if 2 * T > TMAX else 1
                            Tc = 2 * T // ncj
                            hsum = work.tile(
                                [128, 2 * T], f32r, name="hsum", tag="hsum", bufs=3
                            )
                            for cj in range(ncj):
                                cb = choff + 2 * o0 + cj * Tc
                                h2 = Tc // 2
                                hsv = g2(hsum, T)[:, :, cj * h2:(cj + 1) * h2]
                                hb4 = hbu.rearrange("p (k c) -> p k c", k=2)[
                                    :, :, cb:cb + Tc
                                ].rearrange("p k (n two) -> p k n two", two=2)
                                eng = nc.gpsimd if small else nc.vector
                                eng.tensor_add(
                                    hsv, hb4[:, :, :, 0], hb4[:, :, :, 1]
                                )
                            hsums.append(hsum)
                    for j in range(ntile):
                        o0 = j * T
                        xt = None if small else load_x(off, o0, T)
                        cred = None
                        hsum = None
                        pg = close = None
                        u_phase = None
                        if not leaf:
                            ncj = 2 if 2 * T > TMAX else 1
                            Tc = 2 * T // ncj
                            cred = work.tile(
                                [128, 2 * T], f32, name="cred", tag="cred"
                            )
                            hsum = hsums[j]
                            hs_ = hsum
                            u_phase = [
                                (u_bu, lambda k, h=hs_: h[:, k * T:(k + 1) * T])
                            ]
                        if not small:
                            # W-matmuls up front: they only need xt, so PE has
                            # work while the f/hsum chains of this tile run
                            xt_ = xt
                            pg, close = iou_mms(
                                T,
                                [(w_bu, lambda k, x=xt_: x[:, k * T:(k + 1) * T])],
                                u_phase,
                            )
                        if not leaf:
                            for cj in range(ncj):
                                cb = choff + 2 * o0 + cj * Tc
                                pf = psf.tile(
                                    [128, 2 * Tc], f32, name="pf", tag="pf"
                                )
                                for g in (0, 1):
                                    for k in (0, 1):
                                        nc.tensor.matmul(
                                            pf[:, g * Tc:(g + 1) * Tc],
                                            uf_bu[k][:, g * 128:(g + 1) * 128],
                                            hbu[:, k * TOT + cb:k * TOT + cb + Tc],
                                            start=(k == 0),
                                            stop=(k == 1),
                                        )
                                nc.scalar.activation(pf, pf, Sig)
                                # fc = f * c_child (in place in psum)
                                cv = g2(c_next, C_next)[
                                    :, :, 2 * o0 + cj * Tc:2 * o0 + (cj + 1) * Tc
                                ]
                                # fc = f * c_child -> SBUF (DVE cannot read two
                                # PSUM operands, so the pairwise sum needs it
                                # out of PSUM)
                                fct = work.tile(
                                    [128, 2 * Tc], f32, name="fct", tag="fc"
                                )
                                nc.vector.tensor_mul(g2(fct, Tc), g2(pf, Tc), cv)
                                # c_red halves: pairwise sums of fc
                                h2 = Tc // 2
                                crv = g2(cred, T)[:, :, cj * h2:(cj + 1) * h2]
                                fc4 = fct.rearrange(
                                    "p (g n two) -> p g n two", g=2, two=2
                                )
                                # GpSimd is idle; take the cred adds off the
                                # loaded DVE queue (SBUF-only operands)
                                nc.gpsimd.tensor_add(
                                    crv, fc4[:, :, :, 0], fc4[:, :, :, 1]
                                )
                        if small:
                            pg, close = iou_mms(T, u_phase)
                        else:
                            close()
                        cr = None if leaf else ("full", g2(cred, T))
                        gates(
                            pg,
                            T,
                            cr,
                            g2(c_cur, C)[:, :, o0:o0 + T],
                            hbu.rearrange("p (k c) -> p k c", k=2)[
                                :, :, off + o0:off + o0 + T
                            ],
                            pre=pre_bu if small else None,
                            lev=l,
                        )
                    c_next = c_cur
                    C_next = C

            # ---- td precompute: Wx@x + Wh@h_bu over small-level cols ----
            wx_td = load_w(wx_td_d, 3 * H, "wxtd")
            wh_td = load_w(wh_td_d, 3 * H, "whtd")
            u_td = load_w(u_iou_td_d, 3 * H, "utd")
            uf_td = load_w(u_f_td_d, H, "uftd")

            pre_td = precompute(
                [
                    (wx_td, lambda k: xsm[:, k * SM:(k + 1) * SM]),
                    (wh_td, lambda k: hbu[:, k * TOT:k * TOT + SM]),
                ],
            )

            # ================= top-down =================
            with tc.tile_pool(name="td_state", bufs=1) as td_state:
                h_prev = c_prev = None
                C_prev = 0
                for l in range(0, DEPTH + 1):
                    C, off = LEVW[l], LEVO[l]
                    T = min(TMAX, C)
                    leaf = l == DEPTH
                    root = l == 0
                    small = l < SM_LEV
                    par = "A" if l % 2 else "Bp"
                    if not leaf:
                        h_cur = td_state.tile(
                            [128, 2 * C], f32r, name=f"th{l}", tag=f"th{par}"
                        )
                        c_cur = td_state.tile(
                            [128, 2 * C], f32, name=f"tc{l}", tag=f"tc{par}"
                        )
                    else:
                        h_cur = c_cur = None
                    for j in range(C // T):
                        o0 = j * T
                        xt = None if small else load_x(off, o0, T)
                        credp = None
                        pT = T // 2 if not root else 0
                        po = o0 // 2
                        u_phase = None
                        if not root:
                            hp_, po_, pT_, Cp_ = h_prev, po, pT, C_prev
                            u_phase = [
                                (
                                    u_td,
                                    lambda k, h=hp_, a=po_, b=pT_, Cp=Cp_: h[
                                        :, k * Cp + a:k * Cp + a + b
                                    ].to_broadcast([128, b, 2]),
                                )
                            ]
                        if not small:
                            # Wx/Wh matmuls first (inputs all ready)
                            xt_ = xt
                            pg, close = iou_mms(
                                T,
                                [
                                    (wx_td, lambda k, x=xt_: x[:, k * T:(k + 1) * T]),
                                    (
                                        wh_td,
                                        lambda k, a=off + o0: hbu[
                                            :, k * TOT + a:k * TOT + a + T
                                        ],
                                    ),
                                ],
                                u_phase,
                            )
                        if not root:
                            pf = psf.tile([128, 2 * pT], f32, name="pftd", tag="pf")
                            for g in (0, 1):
                                for k in (0, 1):
                                    nc.tensor.matmul(
                                        pf[:, g * pT:(g + 1) * pT],
                                        uf_td[k][:, g * 128:(g + 1) * 128],
                                        h_prev[:, k * C_prev + po:k * C_prev + po + pT],
                                        start=(k == 0),
                                        stop=(k == 1),
                                    )
                            nc.scalar.activation(pf, pf, Sig)
                            credp = work.tile(
                                [128, 2 * pT], f32, name="credp", tag="cred"
                            )
                            nc.vector.tensor_mul(
                                g2(credp, pT),
                                g2(pf, pT),
                                g2(c_prev, C_prev)[:, :, po:po + pT],
                            )
                        if small:
                            # root: no matmuls at all (pre only); other small
                            # levels: U-broadcast matmuls only
                            pg, close = iou_mms(T, u_phase or [])
                        else:
                            close()
                        if root:
                            # iou = pre only: copy into psum
                            for gi, gate in enumerate(("i", "o", "u")):
                                nc.vector.tensor_copy(
                                    g2(pg[gate], T),
                                    pre_td[:, 2 * gi:2 * gi + 2, 0:T],
                                )
                            prearg = None
                        else:
                            prearg = pre_td if small else None
                        if leaf:
                            cl = work.tile([128, 2 * T], f32, name="cl", tag="fc")
                            tree = j

                            def sink(ht, _t=tree):
                                for g in (0, 1):
                                    nc.vector.reduce_sum(
                                        mean[:, g, _t:_t + 1],
                                        ht[:, g * T:(g + 1) * T],
                                        axis=mybir.AxisListType.X,
                                    )

                            gates(
                                pg,
                                T,
                                ("parent", g2(credp, pT)),
                                g2(cl, T),
                                None,
                                leaf_sink=sink,
                                pre=prearg,
                                lev=l,
                            )
                        else:
                            cr = None if root else ("parent", g2(credp, pT))
                            gates(
                                pg,
                                T,
                                cr,
                                g2(c_cur, C)[:, :, o0:o0 + T],
                                g2(h_cur, C)[:, :, o0:o0 + T],
                                pre=prearg,
                                lev=l,
                            )
                    h_prev, c_prev = h_cur, c_cur
                    C_prev = C

            # ---- outputs ----
            mf = mean.rearrange("p g b -> p (g b)")
            nc.vector.tensor_scalar_mul(mf, mf, 1.0 / (1 << DEPTH))
            nc.sync.dma_start(
                out=out_d[0:256, :].rearrange("(k p) b -> p k b", k=2).bitcast(f32r),
                in_=hbu.rearrange("p (k c) -> p k c", k=2)[:, :, 0:bl],
            )
            nc.sync.dma_start(
                out=out_d[256:512, :].rearrange("(g p) b -> p g b", g=2),
                in_=mean,
            )

    if not nc.is_finalized():
        nc.finalize()
    return nc


def _prep_shared(inputs):
    """Weight marshaling shared by all cores (biases are zero by spec)."""
    f = np.ascontiguousarray
    W_iou_td = np.asarray(inputs["W_iou_td"], np.float32)
    return {
        "w_iou_bu_T": f(np.asarray(inputs["W_iou_bu"], np.float32).T),
        "u_iou_bu_T": f(np.asarray(inputs["U_iou_bu"], np.float32).T),
        "u_f_bu_T": f(np.asarray(inputs["U_f_bu"], np.float32).T),
        "wx_iou_td_T": f(W_iou_td[:, :XS].T),
        "wh_iou_td_T": f(W_iou_td[:, XS:].T),
        "u_iou_td_T": f(np.asarray(inputs["U_iou_td"], np.float32).T),
        "u_f_td_T": f(np.asarray(inputs["U_f_td"], np.float32).T),
    }


def prep_xt(Xc):
    """[bl, NN, XS] -> [XS, bl*NN] with level-major column blocks."""
    bl = Xc.shape[0]
    xt = np.asarray(Xc, np.float32).transpose(2, 0, 1)  # [XS, bl, NN]
    blocks = []
    for l in range(DEPTH + 1):
        lo, nl = (1 << l) - 1, 1 << l
        blocks.append(xt[:, :, lo:lo + nl].reshape(XS, bl * nl))
    return np.ascontiguousarray(np.concatenate(blocks, axis=1))


def unpack_out(o, bl):
    """[512, bl] -> [bl, 512] (root_h_bu | leaf mean)."""
    return np.concatenate([o[0:256, :].T, o[256:512, :].T], axis=1)


def kernel(**inputs):
    global LAST_EXEC_NS
    from concourse.bass_utils import run_bass_kernel_spmd

    bl = B // NCORES
    if "nc" not in _CACHE:
        _CACHE["nc"] = _build_nc(bl)
    nc = _CACHE["nc"]

    shared = _prep_shared(inputs)
    X = np.asarray(inputs["X"], np.float32)
    in_maps = []
    for c in range(NCORES):
        m = dict(shared)
        m["xT"] = prep_xt(X[c * bl:(c + 1) * bl])
        in_maps.append(m)

    trace = _CACHE.get("trace", False)
    res = None
    for attempt in range(3):
        try:
            res = run_bass_kernel_spmd(nc, in_maps, list(range(NCORES)), trace=trace)
            break
        except Exception:
            # transient NRT device faults have been observed; retry
            if attempt == 2:
                raise
            import time

            time.sleep(5)
    LAST_EXEC_NS = res.exec_time_ns
    _CACHE["last_results"] = res

    out = np.concatenate(
        [unpack_out(res.results[c]["out"], bl) for c in range(NCORES)], axis=0
    )
    return out.astype(np.float32)

